# revision 11
# baseline (speedup 1.0000x reference)
"""MultiHeadAttention Trainium2 kernel.

Sharding: 8 cores = 2 batches x 4 head-groups (4 heads each).
Each core computes, for its batch b and heads [h0, h0+4):
  qT/kT [256, T] and v [T, 256] from xT @ w_qkv slices (channel-major),
  S^T = k q^T per head ([k, q] layout, causal folded into the mask on host),
  P = exp(S^T/sqrt(D) + maskT), attention out O^T = [v|1]^T P (ones column
  gives the softmax denominators for free), O^T normalized by 1/sums
  (DMA-broadcast across partitions), then the partial output projection
  y_heads @ w_proj[rows]. The host sums the 4 partial projections per batch.

All matmuls run in float32r (single-pass fp32, ~1.5e-4 rel err), softmax in
fp32 on ACT/DVE.
"""

import sys

sys.path.insert(0, "/opt/trn_rl_repo")

import ml_dtypes
import numpy as np

import concourse.bass as bass
import concourse.mybir as mybir
import concourse.tile as tile
from concourse import bacc
from concourse.bass_utils import run_bass_kernel_spmd

B, T, C, H, D = 2, 2048, 1024, 16, 64
HPC = 4  # heads per core
NCORES = 8
KC = C // 128  # 8 contraction chunks for the projections
NT = T // 128  # 16 token tiles
NQ = T // 512  # 4 query slices
F32R, F32, BF16 = mybir.dt.float32r, mybir.dt.float32, mybir.dt.bfloat16
AF = mybir.ActivationFunctionType
NEG = np.float32(-1.0e30)

_cache = {}


def _build():
    nc = bacc.Bacc("TRN2", target_bir_lowering=False, debug=False, num_devices=NCORES)
    xt_d = nc.dram_tensor("xt", [C, T], F32R, kind="ExternalInput")
    maskt_d = nc.dram_tensor("maskt", [T, T], BF16, kind="ExternalInput")
    wqkv_d = nc.dram_tensor("wqkv", [C, 3 * HPC * D], F32R, kind="ExternalInput")
    wproj_d = nc.dram_tensor("wproj", [HPC * D, C], F32R, kind="ExternalInput")
    out_d = nc.dram_tensor("out", [T, C], F32, kind="ExternalOutput")

    with tile.TileContext(nc) as tc:
        with (
            tc.tile_pool(name="ps", bufs=1, space="PSUM") as ps,
            tc.tile_pool(name="apool", bufs=1) as apool,
            tc.tile_pool(name="mpool", bufs=4) as mpool,
            tc.tile_pool(name="espool", bufs=7) as espool,
            tc.tile_pool(name="xpool", bufs=7) as xpool,
            tc.tile_pool(name="spool", bufs=1) as spool,
            tc.tile_pool(name="stpool", bufs=2) as stpool,
            tc.tile_pool(name="dpool", bufs=2, space="DRAM") as dpool,
        ):
          with tc.tile_pool(name="wpool", bufs=1) as wpool:
            # ---- input loads (per-kc chunks so compute can start early)
            xt_sb = wpool.tile([128, KC, T], F32R, tag="xt")
            wqkv_sb = wpool.tile([128, KC, 3 * HPC * D], F32R, tag="wqkv")
            xt_v = xt_d.ap().rearrange("(c p) t -> p c t", p=128)
            wqkv_v = wqkv_d.ap().rearrange("(c p) n -> p c n", p=128)
            for kc in range(KC):
                nc.sync.dma_start(out=wqkv_sb[:, kc, :], in_=wqkv_v[:, kc, :])
                nc.sync.dma_start(out=xt_sb[:, kc, :], in_=xt_v[:, kc, :])

            # ---- phase A: qT, kT (channel-major) and v (token-major, with
            # a ones column appended per head for the softmax denominators)
            qt_tiles, kt_tiles = [], []
            for nm, col0, dst in (("qt", 0, qt_tiles), ("kt", HPC * D, kt_tiles)):
                for m in range(2):
                    t_sb = apool.tile([128, T], BF16, tag=f"{nm}{m}", name=f"{nm}{m}")
                    dst.append(t_sb)
                    for n in range(NQ):
                        qk_ps = ps.tile([128, 2, 512], F32, tag="sp", bufs=2, name="qk_ps")
                        for kc in range(KC):
                            nc.tensor.matmul(
                                qk_ps[:, 0, :],
                                lhsT=wqkv_sb[:, kc, col0 + m * 128 : col0 + (m + 1) * 128],
                                rhs=xt_sb[:, kc, n * 512 : (n + 1) * 512],
                                start=(kc == 0),
                                stop=(kc == KC - 1),
                            )
                        nc.scalar.copy(t_sb[:, n * 512 : (n + 1) * 512], qk_ps[:, 0, :])

            v_sb = apool.tile([128, NT, HPC * 65], BF16, tag="v")
            v_4d = v_sb.rearrange("p t (h e) -> p t h e", h=HPC)
            ones_t = spool.tile([128, NT * HPC], F32, tag="ones", name="ones_t")
            nc.vector.memset(ones_t, 1.0)
            nc.scalar.activation(
                v_4d[:, :, :, 64:65],
                ones_t.rearrange("p (t h one) -> p t h one", t=NT, one=1),
                AF.Copy,
            )
            for tt in range(NT):
                v_ps = ps.tile([128, 2, 512], F32, tag="sp", bufs=2, name="v_ps")
                for kc in range(KC):
                    nc.tensor.matmul(
                        v_ps[:, 0, 0:256],
                        lhsT=xt_sb[:, kc, tt * 128 : (tt + 1) * 128],
                        rhs=wqkv_sb[:, kc, 2 * HPC * D : 3 * HPC * D],
                        start=(kc == 0),
                        stop=(kc == KC - 1),
                    )
                nc.scalar.copy(
                    v_4d[:, tt, :, 0:64],
                    v_ps[:, 0, 0:256].rearrange("p (h d) -> p h d", h=HPC),
                )

          # xt/wqkv are dead now; release their SBUF for the proj weights.
          with tc.tile_pool(name="w2pool", bufs=1) as w2pool:
            wproj_sb = w2pool.tile([128, 2, C], F32R, tag="wproj")
            nc.sync.dma_start(
                out=wproj_sb, in_=wproj_d.ap().rearrange("(m p) n -> p m n", p=128)
            )
            # ---- phase B: attention
            yt_tiles = [
                apool.tile([128, T], F32R, tag=f"yt{m}", name=f"yt{m}") for m in range(2)
            ]
            maskt_v = maskt_d.ap().rearrange("(c p) q -> p c q", p=128)  # [128,16,T]

            def emit_s_group(qs, g):
                """S^T matmuls for group g (2 k-chunks) of q-slice qs, all heads."""
                kc0 = 2 * g
                mt = mpool.tile([128, 2, 512], BF16, tag="mask", name="mt")
                nc.scalar.dma_start(
                    out=mt, in_=maskt_v[:, kc0 : kc0 + 2, qs * 512 : (qs + 1) * 512]
                )
                sps = []
                for h in range(HPC):
                    mh, ph = divmod(h, 2)
                    p0 = ph * 64
                    sp = ps.tile([128, 2, 512], F32, tag="sp", bufs=2, name="sp")
                    for i in range(2):
                        kc = kc0 + i
                        nc.tensor.matmul(
                            sp[:, i, :],
                            lhsT=kt_tiles[mh][p0 : p0 + 64, kc * 128 : (kc + 1) * 128],
                            rhs=qt_tiles[mh][p0 : p0 + 64, qs * 512 : (qs + 1) * 512],
                            start=True,
                            stop=True,
                        )
                    sps.append(sp)
                return mt, sps

            def emit_pv_group(qs, g, pv_all, mt, sps):
                """P = exp(S) * expmask (bf16) + PV matmuls for group g."""
                nkc = 4 * qs + 4
                kc0 = 2 * g
                for h in range(HPC):
                    exps = xpool.tile([128, 2, 512], BF16, tag="exps", name="exps")
                    nc.scalar.activation(exps, sps[h], AF.Exp)
                    es = espool.tile([128, 2, 512], BF16, tag="es", name="es")
                    nc.vector.tensor_mul(es, exps, mt)
                    for i in range(2):
                        kc = kc0 + i
                        nc.tensor.matmul(
                            pv_all[:, h, :],
                            lhsT=v_sb[:, kc, h * 65 : (h + 1) * 65],
                            rhs=es[:, i, :],
                            start=(kc == 0),
                            stop=(kc == nkc - 1),
                        )

            def emit_norm(qs, pv_all):
                """yT = O^T / sums. The 2048 sums sit on one partition (row 64)
                so spread them over 128 partitions via a DRAM bounce before the
                (otherwise lane-serial, 8 cyc/elem) reciprocal."""
                srow = spool.tile([1, HPC, 512], F32, tag="srow", name="srow")
                nc.scalar.copy(srow, pv_all[64:65, :, :])
                d1 = dpool.tile([HPC * 512], F32, tag="d1", name="d1")
                nc.sync.dma_start(out=d1, in_=srow)
                spread = spool.tile([128, 16], F32, tag="spread", name="spread")
                nc.sync.dma_start(out=spread, in_=d1.rearrange("(p e) -> p e", p=128))
                rspread = spool.tile([128, 16], F32, tag="rspread", name="rspread")
                nc.vector.reciprocal(rspread, spread)
                d2 = dpool.tile([HPC * 512], F32, tag="d2", name="d2")
                nc.sync.dma_start(out=d2.rearrange("(p e) -> p e", p=128), in_=rspread)
                bcast = spool.tile([64, HPC, 512], F32, tag="bcast", name="bcast")
                bsrc = bass.AP(
                    tensor=d2.tensor,
                    offset=d2.offset,
                    ap=[[0, 64], [512, HPC], [1, 512]],
                )
                nc.sync.dma_start(out=bcast, in_=bsrc)
                for h in range(HPC):
                    mh, ph = divmod(h, 2)
                    nc.vector.tensor_mul(
                        yt_tiles[mh][ph * 64 : (ph + 1) * 64, qs * 512 : (qs + 1) * 512],
                        pv_all[0:64, h, :],
                        bcast[:, h, :],
                    )

            def emit_proj(qs):
                """partial projection for this q-slice's 4 token tiles."""
                for tt in range(4 * qs, 4 * qs + 4):
                    st = stpool.tile([128, C], F32, tag="stage", name="st")
                    for ns in range(2):
                        pj_ps = ps.tile([128, 2, 512], F32, tag="sp", bufs=2, name="pj_ps")
                        for m in range(2):
                            nc.tensor.matmul(
                                pj_ps[:, 0, :],
                                lhsT=yt_tiles[m][:, tt * 128 : (tt + 1) * 128],
                                rhs=wproj_sb[:, m, ns * 512 : (ns + 1) * 512],
                                start=(m == 0),
                                stop=(m == 1),
                            )
                        nc.vector.tensor_copy(st[:, ns * 512 : (ns + 1) * 512], pj_ps[:, 0, :])
                    nc.sync.dma_start(out=out_d.ap()[tt * 128 : (tt + 1) * 128, :], in_=st)

            # 3-stage software pipeline over all (qs, g) groups:
            #   step i emits  S(i+2)  -> exp/mul(i+1) -> PV(i)
            # so the PE always has the next S matmuls queued while ACT/DVE
            # chew on exp/mul, and PV never waits on the latest exp. The
            # normalization + partial projection of a finished q-slice are
            # deferred one step so their dependency chains hide behind the
            # next slice's matmuls.
            groups = [(qs, g) for qs in range(NQ) for g in range(2 * qs + 2)]
            n = len(groups)
            pv_tiles = {}
            em_out = {}

            def stage_s(i):
                if i < n:
                    em_out[i] = (groups[i], emit_s_group(*groups[i]))

            def stage_em(i):
                if 0 <= i < n:
                    (qs, g), (mt, sps) = em_out[i]
                    tiles = []
                    for h in range(HPC):
                        exps = xpool.tile([128, 2, 512], BF16, tag="exps", name="exps")
                        nc.scalar.activation(exps, sps[h], AF.Exp)
                        es = espool.tile([128, 2, 512], BF16, tag="es", name="es")
                        nc.vector.tensor_mul(es, exps, mt)
                        tiles.append(es)
                    em_out[i] = ((qs, g), tiles)

            def stage_pv(i):
                if not (0 <= i < n):
                    return None
                (qs, g), tiles = em_out.pop(i)
                if g == 0:
                    pv_tiles[qs] = ps.tile(
                        [65, HPC, 512], F32, tag="pv", bufs=1, name="pv_all"
                    )
                nkc = 4 * qs + 4
                kc0 = 2 * g
                for h in range(HPC):
                    for i2 in range(2):
                        kc = kc0 + i2
                        nc.tensor.matmul(
                            pv_tiles[qs][:, h, :],
                            lhsT=v_sb[:, kc, h * 65 : (h + 1) * 65],
                            rhs=tiles[h][:, i2, :],
                            start=(kc == 0),
                            stop=(kc == nkc - 1),
                        )
                if g == 2 * qs + 1:
                    return qs  # q-slice finished
                return None

            stage_s(0)
            stage_s(1)
            stage_em(0)
            done_qs = None
            for i in range(n):
                stage_s(i + 2)
                stage_em(i + 1)
                fin = stage_pv(i)
                if done_qs is not None:
                    emit_norm(done_qs, pv_tiles.pop(done_qs))
                    emit_proj(done_qs)
                done_qs = fin
            if done_qs is not None:
                emit_norm(done_qs, pv_tiles.pop(done_qs))
                emit_proj(done_qs)

    nc.compile()
    return nc


def _get_program():
    if "nc" not in _cache:
        _cache["nc"] = _build()
    return _cache["nc"]


def _prep_in_maps(x, mask, w_qkv, w_proj, head_mask):
    x = np.asarray(x, dtype=np.float32)
    mask = np.asarray(mask, dtype=np.float32)
    w_qkv = np.asarray(w_qkv, dtype=np.float32)
    w_proj = np.asarray(w_proj, dtype=np.float32)
    head_mask = np.asarray(head_mask, dtype=np.float32)

    idx = np.arange(T)
    causal_pen = np.where(idx[:, None] > idx[None, :], NEG, np.float32(0.0))  # [k, q]

    xts, maskts = [], []
    for b in range(B):
        xts.append(np.ascontiguousarray(x[b].T))
        em = np.exp(np.ascontiguousarray(mask[b, 0].T) + causal_pen)
        maskts.append(em.astype(ml_dtypes.bfloat16))

    in_maps = []
    for core in range(NCORES):
        b, hg = divmod(core, NCORES // B)
        h0 = hg * HPC
        wq = w_qkv[:, h0 * D : (h0 + HPC) * D] * np.float32(0.125)  # 1/sqrt(D)
        wk = w_qkv[:, C + h0 * D : C + (h0 + HPC) * D]
        wv = w_qkv[:, 2 * C + h0 * D : 2 * C + (h0 + HPC) * D]
        wqkv_c = np.ascontiguousarray(np.concatenate([wq, wk, wv], axis=1))
        wp = w_proj[h0 * D : (h0 + HPC) * D, :] * np.repeat(head_mask[h0 : h0 + HPC], D)[:, None]
        in_maps.append(
            {
                "xt": xts[b],
                "maskt": maskts[b],
                "wqkv": wqkv_c,
                "wproj": np.ascontiguousarray(wp.astype(np.float32)),
            }
        )
    return in_maps


def run(inputs, trace=False, trace_cores=None):
    nc = _get_program()
    in_maps = _prep_in_maps(**inputs)
    res = run_bass_kernel_spmd(
        nc,
        in_maps,
        list(range(NCORES)),
        trace=trace,
        trace_cores=trace_cores,
    )
    out = np.zeros((B, T, C), dtype=np.float32)
    for core in range(NCORES):
        out[core // (NCORES // B)] += res.results[core]["out"]
    return out, res


def kernel(x, mask, w_qkv, w_proj, head_mask):
    out, _ = run(dict(x=x, mask=mask, w_qkv=w_qkv, w_proj=w_proj, head_mask=head_mask))
    return out
